# revision 17
# baseline (speedup 1.0000x reference)
"""Fastfood layer (nn_BIG_Fastfood_Layer) Trainium2 Bass kernel, v3.

Math (reference):
    xr = x.reshape(2048, 2048)                       # (R, D)
    HBx = fwht_2048(xr * B)                          # (R, D)
    GPHBx[:, j] = HBx[:, P[j] % 2048] * G[j]         # (R, O)
    HG = fwht_8192(GPHBx)                            # (R, O)
    out = cos(HG * S / sqrt(O) + 2*pi*U) * sqrt(2/O)

v3 design (data-parallel rows, 8 cores, rc = 256 rows/core):

  P1: FWHT_2048 = H_128 (x) H_16.  Stage A: 16 fp32r matmuls (lhsT =
      diag(B)H128 per lo).  H_16: level 1 from PSUM (ACT copy + DVE
      add/sub), levels 2/4/8 merged wide adds (DVE) / subs (Pool).
      One DMA writes hbxt (f32, 2 MiB) to DRAM.
  P2: 4 batched dma_gather instructions (2048 idxs each, SWDGE) pull
      feature j = p*64+s to dst[p, s].  G-scale on Pool tensor_scalar;
      stage C = 32 fp32r matmuls (shared lhsT = H128, merged s-pairs).
      PSUM evac to fp32 ct chunks (DVE/ACT alternating).
  TR: SBUF->SBUF DMA realizes the (m,s) -> (s,m-block) transpose
      (8 MiB f32, 1 KiB descriptors), no DRAM round trip.
  P3: stage D = 32 fp32r matmuls (lhsT = blockdiag(H64), merged
      pt-pairs); epilogue z = psd*stab + ubias (DVE TS), k = RNE(z)
      (ACT), d = z - k (DVE), out = Sin(2*pi*d) bf16 (ACT); 32 output
      DMAs.  Host upcasts and applies sqrt(2/O) = 2^-6.
"""

import math

import numpy as np

D = 2048
O = 8192
R = 2048
N_CORES = 8
RC = R // N_CORES  # 256 rows per core

F32R = False  # use fp32r matmuls (4x PE throughput at >=256 moving rows)
SBUF_TRANSPOSE = True  # SBUF->SBUF transpose DMA (else DRAM round trip)

TRACE = False
TRACE_KW = {}

_CACHE = {}


def _hadamard(n):
    h = np.array([[1.0]], dtype=np.float32)
    while h.shape[0] < n:
        h = np.block([[h, h], [h, -h]])
    return h.astype(np.float32)


def _build_nc(rc):
    import concourse.bass as bass
    import concourse.mybir as mybir
    import concourse.tile as tile
    from concourse import bacc

    f32 = mybir.dt.float32
    f32r = mybir.dt.float32r
    bf16 = mybir.dt.bfloat16
    i32 = mybir.dt.int32
    nc = bacc.Bacc("TRN2", target_bir_lowering=False)
    fmm = f32r if F32R else f32

    xt_d = nc.dram_tensor("xt", [16, 128, rc], f32, kind="ExternalInput")
    w1_d = nc.dram_tensor("w1", [128, 16 * 128], f32, kind="ExternalInput")
    h128_d = nc.dram_tensor("h128", [128, 128], f32, kind="ExternalInput")
    hd64_d = nc.dram_tensor("hd64", [128, 128], f32, kind="ExternalInput")
    gtab_d = nc.dram_tensor("gtab", [128, 64], f32, kind="ExternalInput")
    stab_d = nc.dram_tensor("stab", [128, 64], f32, kind="ExternalInput")
    ubias_d = nc.dram_tensor("ubias", [128, 64], f32, kind="ExternalInput")
    idx_d = nc.dram_tensor("idx", [128, 64], i32, kind="ExternalInput")
    gidx_d = nc.dram_tensor("gidx", [128, 512], mybir.dt.int16, kind="ExternalInput")
    outT_d = nc.dram_tensor("outT", [O, rc], bf16, kind="ExternalOutput")

    TWO_PI = float(2.0 * math.pi)

    with tile.TileContext(nc) as tc:
        with (
            tc.tile_pool(name="consts", bufs=1) as cpool,
            tc.tile_pool(name="p1", bufs=1) as p1pool,
            tc.tile_pool(name="g", bufs=2) as gpool,
            tc.tile_pool(name="p32", bufs=4) as p32pool,
            tc.tile_pool(name="ct", bufs=2) as ctpool,
            tc.tile_pool(name="dt", bufs=1) as dtpool,
            tc.tile_pool(name="ep", bufs=2) as eppool,
            tc.tile_pool(name="ps", bufs=4, space="PSUM") as pspool,
            tc.tile_pool(name="dram", bufs=1, space="DRAM") as drampool,
        ):
            ldeng = nc.gpsimd if F32R else nc.sync
            w1 = cpool.tile([128, 16 * 128], fmm)
            ldeng.dma_start(w1[:], w1_d[:])
            h128 = cpool.tile([128, 128], fmm)
            ldeng.dma_start(h128[:], h128_d[:])
            hd64 = cpool.tile([128, 128], fmm)
            ldeng.dma_start(hd64[:], hd64_d[:])
            gtab = cpool.tile([128, 64], f32)
            nc.sync.dma_start(gtab[:], gtab_d[:])
            stab = cpool.tile([128, 64], f32)
            nc.sync.dma_start(stab[:], stab_d[:])
            ubias = cpool.tile([128, 64], f32)
            nc.sync.dma_start(ubias[:], ubias_d[:])
            idx = cpool.tile([128, 64], i32)
            nc.sync.dma_start(idx[:], idx_d[:])
            gidx = cpool.tile([128, 512], mybir.dt.int16)
            nc.sync.dma_start(gidx[:], gidx_d[:])

            hbxt = drampool.tile([D, rc], f32)

            # ---- P1: load x (one DMA), stage A, H16 butterfly ----
            xbuf = p1pool.tile([128, 16 * rc], fmm)
            ldeng.dma_start(
                xbuf[:],
                xt_d[:, :, :].rearrange("l p r -> p l r"),
            )

            abuf = p1pool.tile([128, 16 * rc], f32, tag="a")
            bbuf = p1pool.tile([128, 16 * rc], f32, tag="b")
            for q in range(8):
                ps = pspool.tile([128, 2 * rc], f32, tag="ps")
                for s in range(2):
                    lo = 2 * q + s
                    nc.tensor.matmul(
                        out=ps[:, s * rc : (s + 1) * rc],
                        lhsT=w1[:, lo * 128 : (lo + 1) * 128],
                        rhs=xbuf[:, lo * rc : (lo + 1) * rc],
                        start=True,
                        stop=True,
                    )
                # H16 level h=1 (TT reads at most one PSUM input)
                odd = bbuf[:, (2 * q) * rc : (2 * q + 1) * rc]
                nc.scalar.activation(
                    out=odd,
                    in_=ps[:, rc : 2 * rc],
                    func=mybir.ActivationFunctionType.Copy,
                )
                nc.vector.tensor_tensor(
                    out=abuf[:, (2 * q) * rc : (2 * q + 1) * rc],
                    in0=ps[:, 0:rc],
                    in1=odd,
                    op=mybir.AluOpType.add,
                )
                nc.vector.tensor_tensor(
                    out=abuf[:, (2 * q + 1) * rc : (2 * q + 2) * rc],
                    in0=ps[:, 0:rc],
                    in1=odd,
                    op=mybir.AluOpType.subtract,
                )
            src, dst = abuf, bbuf
            for h in (2, 4, 8):
                w = h * rc
                for i in range(0, 16, 2 * h):
                    a0 = src[:, i * rc : i * rc + w]
                    a1 = src[:, (i + h) * rc : (i + h) * rc + w]
                    nc.vector.tensor_tensor(
                        out=dst[:, i * rc : i * rc + w],
                        in0=a0, in1=a1, op=mybir.AluOpType.add,
                    )
                    nc.vector.tensor_tensor(
                        out=dst[:, (i + h) * rc : (i + h) * rc + w],
                        in0=a0, in1=a1, op=mybir.AluOpType.subtract,
                    )
                src, dst = dst, src
            hb = src  # hb[hi, lo*rc+r] = HBx[r, hi*16+lo]

            # hbxt row lo*128 + hi
            nc.sync.dma_start(
                hbxt[:].rearrange("(l h) r -> h l r", l=16),
                hb[:],
            )

            # ---- P2: batched gathers + G-scale + stage C ----
            # feature j = p*64 + s  ->  gt[p, s_local, r]
            dtf = dtpool.tile([128, 64 * rc], fmm)
            ctd = drampool.tile([O, rc], fmm)  # row (b*64+s)*64 + pt
            ctd4 = ctd[:].rearrange("(b s pt) r -> b pt s r", b=2, pt=64)
            for g4 in range(4):
                gt = gpool.tile([128, 16, rc], f32, tag="gt")
                for hh in range(2):  # 1024 idxs per instr (HW SWDGE limit)
                    nc.gpsimd.dma_gather(
                        out_ap=gt[:, 8 * hh : 8 * (hh + 1), :],
                        in_ap=hbxt[:, :],
                        idxs_ap=gidx[
                            :, 128 * g4 + 64 * hh : 128 * g4 + 64 * (hh + 1)
                        ],
                        num_idxs=1024,
                        num_idxs_reg=1024,
                        elem_size=rc,
                    )
                ctc = ctpool.tile([128, 16 * rc], fmm, tag="ct")
                for t in range(8):  # s-pairs within the chunk
                    s0 = 2 * t
                    p32 = p32pool.tile([128, 2 * rc], fmm, tag="p32")
                    for s in range(2):
                        sg = 16 * g4 + s0 + s
                        if s == 0:
                            nc.vector.tensor_scalar(
                                out=p32[:, s * rc : (s + 1) * rc],
                                in0=gt[:, s0 + s, :],
                                scalar1=gtab[:, sg : sg + 1],
                                scalar2=None,
                                op0=mybir.AluOpType.mult,
                            )
                        else:
                            nc.scalar.activation(
                                out=p32[:, s * rc : (s + 1) * rc],
                                in_=gt[:, s0 + s, :],
                                func=mybir.ActivationFunctionType.Copy,
                                scale=gtab[:, sg : sg + 1],
                            )
                    psc = pspool.tile([128, 2 * rc], f32, tag="ps")
                    nc.tensor.matmul(
                        out=psc[:],
                        lhsT=h128[:],
                        rhs=p32[:],
                        start=True,
                        stop=True,
                    )
                    # evacuate PSUM -> ct chunk (alternate DVE/ACT)
                    dst_sl = ctc[:, s0 * rc : (s0 + 2) * rc]
                    if t % 2 == 0:
                        nc.vector.tensor_copy(out=dst_sl, in_=psc[:])
                    else:
                        nc.scalar.activation(
                            out=dst_sl,
                            in_=psc[:],
                            func=mybir.ActivationFunctionType.Copy,
                        )
                # transpose via DRAM: ctd row (b*64+s)*64 + pt holds
                # ct[m = b*64+pt, s, :]; SBUF side plain, DRAM side fancy
                for b in range(2):
                    nc.sync.dma_start(
                        ctd4[b, :, 16 * g4 : 16 * (g4 + 1), :],
                        ctc[b * 64 : (b + 1) * 64, :].rearrange(
                            "pt (s r) -> pt s r", s=16
                        ),
                    )

            # read back plain: per partition q, rows [q*64, q*64+64)
            ctd_rd = ctd[:].rearrange("(q x) r -> q (x r)", x=64)
            for c4 in range(4):
                nc.sync.dma_start(
                    dtf[:, c4 * 16 * rc : (c4 + 1) * 16 * rc],
                    ctd_rd[:, c4 * 16 * rc : (c4 + 1) * 16 * rc],
                )

            # ---- P3: stage D + epilogue ----
            # psd partition q2 = b*64+s2 <-> feature j2 = (b*64+pt)*64+s2
            outv = outT_d[:].rearrange(
                "(b k t s) r -> k b s t r", b=2, t=2, s=64
            )
            for k in range(32):
                psd = pspool.tile([128, 2 * rc], f32, tag="ps")
                nc.tensor.matmul(
                    out=psd[:],
                    lhsT=hd64[:],
                    rhs=dtf[:, (2 * k) * rc : (2 * k + 2) * rc],
                    start=True,
                    stop=True,
                )
                z = eppool.tile([128, 2 * rc], f32, tag="z")
                for t in range(2):
                    pt = 2 * k + t
                    if t == 0:
                        nc.vector.tensor_scalar(
                            out=z[:, t * rc : (t + 1) * rc],
                            in0=psd[:, t * rc : (t + 1) * rc],
                            scalar1=stab[:, pt : pt + 1],
                            scalar2=ubias[:, pt : pt + 1],
                            op0=mybir.AluOpType.mult,
                            op1=mybir.AluOpType.add,
                        )
                    else:
                        nc.scalar.activation(
                            out=z[:, t * rc : (t + 1) * rc],
                            in_=psd[:, t * rc : (t + 1) * rc],
                            func=mybir.ActivationFunctionType.Identity,
                            scale=stab[:, pt : pt + 1],
                            bias=ubias[:, pt : pt + 1],
                        )
                k32 = eppool.tile([128, 2 * rc], i32, tag="k")
                nc.scalar.activation(
                    out=k32[:],
                    in_=z[:],
                    func=mybir.ActivationFunctionType.Copy,
                )
                nc.vector.tensor_tensor(
                    out=z[:], in0=z[:], in1=k32[:],
                    op=mybir.AluOpType.subtract,
                )
                so = eppool.tile([128, 2 * rc], bf16, tag="so")
                nc.scalar.activation(
                    out=so[:],
                    in_=z[:],
                    func=mybir.ActivationFunctionType.Sin,
                    scale=TWO_PI,
                )
                so3 = so[:].rearrange("q (t r) -> q t r", t=2)
                for b in range(2):
                    nc.sync.dma_start(
                        outv[k, b],
                        so3[b * 64 : (b + 1) * 64, :, :],
                    )

    nc.compile()
    return nc


def host_prep(x, B, G, S, P, U):
    xr = np.ascontiguousarray(x.reshape(R, D).astype(np.float32))
    H128 = _hadamard(128)
    H64 = _hadamard(64)

    # w1[hi, lo*128 + m] = B[hi*16+lo] * H128[hi, m]
    Bm = B.reshape(128, 16).astype(np.float32)
    w1 = (Bm[:, :, None] * H128[:, None, :]).reshape(128, 16 * 128)
    hd64 = np.zeros((128, 128), dtype=np.float32)
    hd64[:64, :64] = H64
    hd64[64:, 64:] = H64

    # gather index order: i = s*128 + p covers feature j = p*64 + s
    jp, js = np.meshgrid(np.arange(128), np.arange(64), indexing="ij")
    jmat = jp * 64 + js  # [p, s]
    fp = (P.astype(np.int64) % D)[jmat]  # [p, s] source feature
    hrow = (fp % 16) * 128 + fp // 16  # hbxt row = lo*128 + hi
    idx = np.ascontiguousarray(hrow.astype(np.int32))  # [p, s]
    # dma_gather index table: i = s*128 + p -> [i%16, i//16], replicated
    # across the 8 GpSimd DSP cores (16-partition stripes)
    idx_lin = np.empty(O, dtype=np.int16)
    i_idx = js * 128 + jp  # [p, s]
    idx_lin[i_idx.ravel()] = hrow.astype(np.int16).ravel()
    gidx = np.ascontiguousarray(np.tile(idx_lin.reshape(512, 16).T, (8, 1)))

    # G at gather layout: gtab[p, s] = G[p*64+s]
    gtab = np.ascontiguousarray(G[jmat].astype(np.float32))

    # final feature j2 = (b*64+pt)*64 + s2 at psd[q2 = b*64+s2, tile pt]
    qb, qpt = np.meshgrid(np.arange(128), np.arange(64), indexing="ij")
    j2 = ((qb // 64) * 64 + qpt) * 64 + (qb % 64)  # [q2, pt]
    s_sc = (S.astype(np.float64) / (math.sqrt(O) * 2.0 * math.pi)).astype(
        np.float32
    )
    ub = (U.astype(np.float64) + 0.25).astype(np.float32)
    stab = np.ascontiguousarray(s_sc[j2])
    ubias = np.ascontiguousarray(ub[j2])

    consts = dict(
        w1=np.ascontiguousarray(w1),
        h128=np.ascontiguousarray(H128),
        hd64=hd64,
        gtab=gtab,
        stab=stab,
        ubias=ubias,
        idx=idx,
        gidx=gidx,
    )

    shards = []
    for c in range(N_CORES):
        xs = xr[c * RC : (c + 1) * RC]  # (RC, D)
        xt = np.ascontiguousarray(
            xs.T.reshape(128, 16, RC).transpose(1, 0, 2)
        )  # xt[lo, hi, r] = xs[r, hi*16+lo]
        shards.append(xt)
    return consts, shards


def assemble(core_outs):
    """core_outs: list of (O, RC) bf16 arrays -> full (R, O) f32 output."""
    out = np.empty((R, O), dtype=np.float32)
    for c, ot in enumerate(core_outs):
        out[c * RC : (c + 1) * RC, :] = ot.astype(np.float32).T
    out *= np.float32(1.0 / 64.0)  # sqrt(2/O) = 2^-6, exact
    return out


def kernel(x, B, G, S, P, U):
    from concourse.bass_utils import run_bass_kernel_spmd

    if "nc" not in _CACHE:
        _CACHE["nc"] = _build_nc(RC)
    nc = _CACHE["nc"]

    consts, shards = host_prep(x, B, G, S, P, U)
    in_maps = [dict(consts, xt=shards[c]) for c in range(N_CORES)]

    res = run_bass_kernel_spmd(
        nc,
        in_maps,
        core_ids=list(range(N_CORES)),
        trace=TRACE,
        **TRACE_KW,
    )
    _CACHE["last_result"] = res
    return assemble([r["outT"] for r in res.results])


# revision 19
# speedup vs baseline: 1.0761x; 1.0761x over previous
"""Fastfood layer (nn_BIG_Fastfood_Layer) Trainium2 Bass kernel, v3.

Math (reference):
    xr = x.reshape(2048, 2048)                       # (R, D)
    HBx = fwht_2048(xr * B)                          # (R, D)
    GPHBx[:, j] = HBx[:, P[j] % 2048] * G[j]         # (R, O)
    HG = fwht_8192(GPHBx)                            # (R, O)
    out = cos(HG * S / sqrt(O) + 2*pi*U) * sqrt(2/O)

v3 design (data-parallel rows, 8 cores, rc = 256 rows/core):

  P1: FWHT_2048 = H_128 (x) H_16.  Stage A: 16 fp32r matmuls (lhsT =
      diag(B)H128 per lo).  H_16: level 1 from PSUM (ACT copy + DVE
      add/sub), levels 2/4/8 merged wide adds (DVE) / subs (Pool).
      One DMA writes hbxt (f32, 2 MiB) to DRAM.
  P2: 4 batched dma_gather instructions (2048 idxs each, SWDGE) pull
      feature j = p*64+s to dst[p, s].  G-scale on Pool tensor_scalar;
      stage C = 32 fp32r matmuls (shared lhsT = H128, merged s-pairs).
      PSUM evac to fp32 ct chunks (DVE/ACT alternating).
  TR: SBUF->SBUF DMA realizes the (m,s) -> (s,m-block) transpose
      (8 MiB f32, 1 KiB descriptors), no DRAM round trip.
  P3: stage D = 32 fp32r matmuls (lhsT = blockdiag(H64), merged
      pt-pairs); epilogue z = psd*stab + ubias (DVE TS), k = RNE(z)
      (ACT), d = z - k (DVE), out = Sin(2*pi*d) bf16 (ACT); 32 output
      DMAs.  Host upcasts and applies sqrt(2/O) = 2^-6.
"""

import math

import numpy as np

D = 2048
O = 8192
R = 2048
N_CORES = 8
RC = R // N_CORES  # 256 rows per core

F32R = False  # use fp32r matmuls (4x PE throughput at >=256 moving rows)
SBUF_TRANSPOSE = True  # SBUF->SBUF transpose DMA (else DRAM round trip)

TRACE = False
TRACE_KW = {}

_CACHE = {}


def _hadamard(n):
    h = np.array([[1.0]], dtype=np.float32)
    while h.shape[0] < n:
        h = np.block([[h, h], [h, -h]])
    return h.astype(np.float32)


def _build_nc(rc):
    import concourse.bass as bass
    import concourse.mybir as mybir
    import concourse.tile as tile
    from concourse import bacc

    f32 = mybir.dt.float32
    f32r = mybir.dt.float32r
    bf16 = mybir.dt.bfloat16
    i32 = mybir.dt.int32
    nc = bacc.Bacc("TRN2", target_bir_lowering=False)
    fmm = f32r if F32R else f32

    xt_d = nc.dram_tensor("xt", [16, 128, rc], f32, kind="ExternalInput")
    w1_d = nc.dram_tensor("w1", [128, 16 * 128], f32, kind="ExternalInput")
    h128_d = nc.dram_tensor("h128", [128, 128], f32, kind="ExternalInput")
    hd64_d = nc.dram_tensor("hd64", [128, 128], f32, kind="ExternalInput")
    gtab_d = nc.dram_tensor("gtab", [128, 64], f32, kind="ExternalInput")
    stab_d = nc.dram_tensor("stab", [128, 64], f32, kind="ExternalInput")
    ubias_d = nc.dram_tensor("ubias", [128, 64], f32, kind="ExternalInput")
    idx_d = nc.dram_tensor("idx", [128, 64], i32, kind="ExternalInput")
    gidx_d = nc.dram_tensor("gidx", [128, 512], mybir.dt.int16, kind="ExternalInput")
    outT_d = nc.dram_tensor("outT", [O, rc], bf16, kind="ExternalOutput")

    TWO_PI = float(2.0 * math.pi)

    with tile.TileContext(nc) as tc:
        with (
            tc.tile_pool(name="consts", bufs=1) as cpool,
            tc.tile_pool(name="p1", bufs=1) as p1pool,
            tc.tile_pool(name="g", bufs=2) as gpool,
            tc.tile_pool(name="p32", bufs=4) as p32pool,
            tc.tile_pool(name="ct", bufs=2) as ctpool,
            tc.tile_pool(name="dt", bufs=1) as dtpool,
            tc.tile_pool(name="ep", bufs=2) as eppool,
            tc.tile_pool(name="ps", bufs=4, space="PSUM") as pspool,
            tc.tile_pool(name="dram", bufs=1, space="DRAM") as drampool,
        ):
            ldeng = nc.gpsimd if F32R else nc.sync
            w1 = cpool.tile([128, 16 * 128], fmm)
            ldeng.dma_start(w1[:], w1_d[:])
            h128 = cpool.tile([128, 128], fmm)
            ldeng.dma_start(h128[:], h128_d[:])
            hd64 = cpool.tile([128, 128], fmm)
            ldeng.dma_start(hd64[:], hd64_d[:])
            gtab = cpool.tile([128, 64], f32)
            nc.sync.dma_start(gtab[:], gtab_d[:])
            stab = cpool.tile([128, 64], f32)
            nc.sync.dma_start(stab[:], stab_d[:])
            ubias = cpool.tile([128, 64], f32)
            nc.sync.dma_start(ubias[:], ubias_d[:])
            idx = cpool.tile([128, 64], i32)
            nc.sync.dma_start(idx[:], idx_d[:])
            gidx = cpool.tile([128, 512], mybir.dt.int16)
            nc.sync.dma_start(gidx[:], gidx_d[:])

            hbxt = drampool.tile([D, rc], f32)

            # ---- P1: load x (one DMA), stage A, H16 butterfly ----
            xbuf = p1pool.tile([128, 16 * rc], fmm)
            for lc in range(4):
                ldeng.dma_start(
                    xbuf[:, 4 * lc * rc : 4 * (lc + 1) * rc],
                    xt_d[4 * lc : 4 * (lc + 1), :, :].rearrange(
                        "l p r -> p l r"
                    ),
                )

            abuf = p1pool.tile([128, 16 * rc], f32, tag="a")
            bbuf = p1pool.tile([128, 16 * rc], f32, tag="b")
            for q in range(8):
                ps = pspool.tile([128, 2 * rc], f32, tag="ps")
                for s in range(2):
                    lo = 2 * q + s
                    nc.tensor.matmul(
                        out=ps[:, s * rc : (s + 1) * rc],
                        lhsT=w1[:, lo * 128 : (lo + 1) * 128],
                        rhs=xbuf[:, lo * rc : (lo + 1) * rc],
                        start=True,
                        stop=True,
                    )
                # H16 level h=1 (TT reads at most one PSUM input)
                odd = bbuf[:, (2 * q) * rc : (2 * q + 1) * rc]
                nc.scalar.activation(
                    out=odd,
                    in_=ps[:, rc : 2 * rc],
                    func=mybir.ActivationFunctionType.Copy,
                )
                nc.vector.tensor_tensor(
                    out=abuf[:, (2 * q) * rc : (2 * q + 1) * rc],
                    in0=ps[:, 0:rc],
                    in1=odd,
                    op=mybir.AluOpType.add,
                )
                nc.vector.tensor_tensor(
                    out=abuf[:, (2 * q + 1) * rc : (2 * q + 2) * rc],
                    in0=ps[:, 0:rc],
                    in1=odd,
                    op=mybir.AluOpType.subtract,
                )
            src, dst = abuf, bbuf
            for h in (2, 4, 8):
                w = h * rc
                sub_eng = nc.vector if h == 4 else nc.gpsimd
                for i in range(0, 16, 2 * h):
                    a0 = src[:, i * rc : i * rc + w]
                    a1 = src[:, (i + h) * rc : (i + h) * rc + w]
                    nc.vector.tensor_tensor(
                        out=dst[:, i * rc : i * rc + w],
                        in0=a0, in1=a1, op=mybir.AluOpType.add,
                    )
                    sub_eng.tensor_tensor(
                        out=dst[:, (i + h) * rc : (i + h) * rc + w],
                        in0=a0, in1=a1, op=mybir.AluOpType.subtract,
                    )
                src, dst = dst, src
            hb = src  # hb[hi, lo*rc+r] = HBx[r, hi*16+lo]

            # hbxt row lo*128 + hi
            nc.sync.dma_start(
                hbxt[:].rearrange("(l h) r -> h l r", l=16),
                hb[:],
            )

            # ---- P2: batched gathers + G-scale + stage C ----
            # feature j = p*64 + s  ->  gt[p, s_local, r]
            dtf = dtpool.tile([128, 64 * rc], fmm)
            ctd = drampool.tile([O, rc], fmm)  # row (b*64+s)*64 + pt
            ctd4 = ctd[:].rearrange("(b s pt) r -> b pt s r", b=2, pt=64)
            for g4 in range(4):
                gt = gpool.tile([128, 16, rc], f32, tag="gt")
                for sl in range(16):
                    sg = 16 * g4 + sl
                    nc.gpsimd.indirect_dma_start(
                        out=gt[:, sl, :],
                        out_offset=None,
                        in_=hbxt[:, :],
                        in_offset=bass.IndirectOffsetOnAxis(
                            ap=idx[:, sg : sg + 1], axis=0
                        ),
                    )
                ctc = ctpool.tile([128, 16 * rc], fmm, tag="ct")
                for t in range(8):  # s-pairs within the chunk
                    s0 = 2 * t
                    p32 = p32pool.tile([128, 2 * rc], fmm, tag="p32")
                    for s in range(2):
                        sg = 16 * g4 + s0 + s
                        if s == 0:
                            nc.vector.tensor_scalar(
                                out=p32[:, s * rc : (s + 1) * rc],
                                in0=gt[:, s0 + s, :],
                                scalar1=gtab[:, sg : sg + 1],
                                scalar2=None,
                                op0=mybir.AluOpType.mult,
                            )
                        else:
                            nc.scalar.activation(
                                out=p32[:, s * rc : (s + 1) * rc],
                                in_=gt[:, s0 + s, :],
                                func=mybir.ActivationFunctionType.Copy,
                                scale=gtab[:, sg : sg + 1],
                            )
                    psc = pspool.tile([128, 2 * rc], f32, tag="ps")
                    nc.tensor.matmul(
                        out=psc[:],
                        lhsT=h128[:],
                        rhs=p32[:],
                        start=True,
                        stop=True,
                    )
                    # evacuate PSUM -> ct chunk (alternate DVE/ACT)
                    dst_sl = ctc[:, s0 * rc : (s0 + 2) * rc]
                    if t % 2 == 0:
                        nc.vector.tensor_copy(out=dst_sl, in_=psc[:])
                    else:
                        nc.scalar.activation(
                            out=dst_sl,
                            in_=psc[:],
                            func=mybir.ActivationFunctionType.Copy,
                        )
                # transpose via DRAM: ctd row (b*64+s)*64 + pt holds
                # ct[m = b*64+pt, s, :]; SBUF side plain, DRAM side fancy
                for b in range(2):
                    nc.sync.dma_start(
                        ctd4[b, :, 16 * g4 : 16 * (g4 + 1), :],
                        ctc[b * 64 : (b + 1) * 64, :].rearrange(
                            "pt (s r) -> pt s r", s=16
                        ),
                    )

            # read back plain: per partition q, rows [q*64, q*64+64)
            ctd_rd = ctd[:].rearrange("(q x) r -> q (x r)", x=64)
            for c4 in range(4):
                nc.sync.dma_start(
                    dtf[:, c4 * 16 * rc : (c4 + 1) * 16 * rc],
                    ctd_rd[:, c4 * 16 * rc : (c4 + 1) * 16 * rc],
                )

            # ---- P3: stage D + epilogue ----
            # psd partition q2 = b*64+s2 <-> feature j2 = (b*64+pt)*64+s2
            outv = outT_d[:].rearrange(
                "(b k t s) r -> k b s t r", b=2, t=2, s=64
            )
            for k in range(32):
                psd = pspool.tile([128, 2 * rc], f32, tag="ps")
                nc.tensor.matmul(
                    out=psd[:],
                    lhsT=hd64[:],
                    rhs=dtf[:, (2 * k) * rc : (2 * k + 2) * rc],
                    start=True,
                    stop=True,
                )
                z = eppool.tile([128, 2 * rc], f32, tag="z")
                for t in range(2):
                    pt = 2 * k + t
                    nc.vector.tensor_scalar(
                        out=z[:, t * rc : (t + 1) * rc],
                        in0=psd[:, t * rc : (t + 1) * rc],
                        scalar1=stab[:, pt : pt + 1],
                        scalar2=ubias[:, pt : pt + 1],
                        op0=mybir.AluOpType.mult,
                        op1=mybir.AluOpType.add,
                    )
                k32 = eppool.tile([128, 2 * rc], i32, tag="k")
                nc.scalar.activation(
                    out=k32[:],
                    in_=z[:],
                    func=mybir.ActivationFunctionType.Copy,
                )
                nc.vector.tensor_tensor(
                    out=z[:], in0=z[:], in1=k32[:],
                    op=mybir.AluOpType.subtract,
                )
                so = eppool.tile([128, 2 * rc], bf16, tag="so")
                nc.scalar.activation(
                    out=so[:],
                    in_=z[:],
                    func=mybir.ActivationFunctionType.Sin,
                    scale=TWO_PI,
                )
                so3 = so[:].rearrange("q (t r) -> q t r", t=2)
                for b in range(2):
                    nc.sync.dma_start(
                        outv[k, b],
                        so3[b * 64 : (b + 1) * 64, :, :],
                    )

    nc.compile()
    return nc


def host_prep(x, B, G, S, P, U):
    xr = np.ascontiguousarray(x.reshape(R, D).astype(np.float32))
    H128 = _hadamard(128)
    H64 = _hadamard(64)

    # w1[hi, lo*128 + m] = B[hi*16+lo] * H128[hi, m]
    Bm = B.reshape(128, 16).astype(np.float32)
    w1 = (Bm[:, :, None] * H128[:, None, :]).reshape(128, 16 * 128)
    hd64 = np.zeros((128, 128), dtype=np.float32)
    hd64[:64, :64] = H64
    hd64[64:, 64:] = H64

    # gather index order: i = s*128 + p covers feature j = p*64 + s
    jp, js = np.meshgrid(np.arange(128), np.arange(64), indexing="ij")
    jmat = jp * 64 + js  # [p, s]
    fp = (P.astype(np.int64) % D)[jmat]  # [p, s] source feature
    hrow = (fp % 16) * 128 + fp // 16  # hbxt row = lo*128 + hi
    idx = np.ascontiguousarray(hrow.astype(np.int32))  # [p, s]
    # dma_gather index table: i = s*128 + p -> [i%16, i//16], replicated
    # across the 8 GpSimd DSP cores (16-partition stripes)
    idx_lin = np.empty(O, dtype=np.int16)
    i_idx = js * 128 + jp  # [p, s]
    idx_lin[i_idx.ravel()] = hrow.astype(np.int16).ravel()
    gidx = np.ascontiguousarray(np.tile(idx_lin.reshape(512, 16).T, (8, 1)))

    # G at gather layout: gtab[p, s] = G[p*64+s]
    gtab = np.ascontiguousarray(G[jmat].astype(np.float32))

    # final feature j2 = (b*64+pt)*64 + s2 at psd[q2 = b*64+s2, tile pt]
    qb, qpt = np.meshgrid(np.arange(128), np.arange(64), indexing="ij")
    j2 = ((qb // 64) * 64 + qpt) * 64 + (qb % 64)  # [q2, pt]
    s_sc = (S.astype(np.float64) / (math.sqrt(O) * 2.0 * math.pi)).astype(
        np.float32
    )
    ub = (U.astype(np.float64) + 0.25).astype(np.float32)
    stab = np.ascontiguousarray(s_sc[j2])
    ubias = np.ascontiguousarray(ub[j2])

    consts = dict(
        w1=np.ascontiguousarray(w1),
        h128=np.ascontiguousarray(H128),
        hd64=hd64,
        gtab=gtab,
        stab=stab,
        ubias=ubias,
        idx=idx,
        gidx=gidx,
    )

    shards = []
    for c in range(N_CORES):
        xs = xr[c * RC : (c + 1) * RC]  # (RC, D)
        xt = np.ascontiguousarray(
            xs.T.reshape(128, 16, RC).transpose(1, 0, 2)
        )  # xt[lo, hi, r] = xs[r, hi*16+lo]
        shards.append(xt)
    return consts, shards


def assemble(core_outs):
    """core_outs: list of (O, RC) bf16 arrays -> full (R, O) f32 output."""
    out = np.empty((R, O), dtype=np.float32)
    for c, ot in enumerate(core_outs):
        out[c * RC : (c + 1) * RC, :] = ot.astype(np.float32).T
    out *= np.float32(1.0 / 64.0)  # sqrt(2/O) = 2^-6, exact
    return out


def kernel(x, B, G, S, P, U):
    from concourse.bass_utils import run_bass_kernel_spmd

    if "nc" not in _CACHE:
        _CACHE["nc"] = _build_nc(RC)
    nc = _CACHE["nc"]

    consts, shards = host_prep(x, B, G, S, P, U)
    in_maps = [dict(consts, xt=shards[c]) for c in range(N_CORES)]

    res = run_bass_kernel_spmd(
        nc,
        in_maps,
        core_ids=list(range(N_CORES)),
        trace=TRACE,
        **TRACE_KW,
    )
    _CACHE["last_result"] = res
    return assemble([r["outT"] for r in res.results])


# revision 21
# speedup vs baseline: 1.2363x; 1.1489x over previous
"""Fastfood layer (nn_BIG_Fastfood_Layer) Trainium2 Bass kernel, v2.

Math (reference):
    xr = x.reshape(2048, 2048)                       # (R, D)
    HBx = fwht_2048(xr * B)                          # (R, D)
    GPHBx[:, j] = HBx[:, P[j] % 2048] * G[j]         # (R, O) tile+permute+scale
    HG = fwht_8192(GPHBx)                            # (R, O)
    out = cos(HG * S / sqrt(O) + 2*pi*U) * sqrt(2/O) # (R, O)

v2 design (data-parallel rows, 8 cores, rc = 256 rows/core; feature-major):

  P1: one DMA loads x transposed; FWHT_2048 = H_128 (x) H_16:
      stage A = 16 fp32 matmuls (lhsT = diag(B_lo) @ H128) packed in pairs
      into [128,512] PSUM banks; H_16 butterfly: level 1 reads PSUM pairs,
      levels 2/4/8 merged wide ops, add/sub split DVE/Pool; one DMA writes
      hbxt to DRAM rows lo*128 + hi.
  P2: 64 single-index indirect gathers (Pool SWDGE paces this phase)
      realize the P-permutation + x4 tile + retile; G-scale per tile
      (ACT/DVE); stage C = 64 fp32 matmuls lhsT = H128 (PE hidden under
      Pool); PSUM pairs split to bf16 hi/lo interleaved in ctbuf
      (hi-copy on ACT, lo = psum - hi on DVE).
  P3: linear DRAM round trip realizes the flip; stage D per tile two
      -> dt (part = lo + 64*(hi'&1), tile=hi'>>1); stage D = per tile two
      bf16 matmuls (hi + lo PSUM-accumulated) with lhsT = blockdiag(H64);
      epilogue z = psum*stab + ubias (TS per tile), k = int32(z) RNE,
      d = z - k, out = Sin(2*pi*d) in bf16; host up-casts and applies
      sqrt(2/O) = 2^-6 (exact).
"""

import math

import numpy as np

D = 2048
O = 8192
R = 2048
N_CORES = 8
RC = R // N_CORES  # 256 rows per core

TRACE = False
TRACE_KW = {}

_CACHE = {}


def _hadamard(n):
    h = np.array([[1.0]], dtype=np.float32)
    while h.shape[0] < n:
        h = np.block([[h, h], [h, -h]])
    return h.astype(np.float32)


def _build_nc(rc):
    import concourse.bass as bass
    import concourse.mybir as mybir
    import concourse.tile as tile
    from concourse import bacc

    f32 = mybir.dt.float32
    bf16 = mybir.dt.bfloat16
    i32 = mybir.dt.int32
    nc = bacc.Bacc("TRN2", target_bir_lowering=False)

    xt_d = nc.dram_tensor("xt", [16, 128, rc], f32, kind="ExternalInput")
    w1_d = nc.dram_tensor("w1", [128, 16 * 128], f32, kind="ExternalInput")
    h128f_d = nc.dram_tensor("h128", [128, 128], f32, kind="ExternalInput")
    hd64_d = nc.dram_tensor("hd64", [128, 128], bf16, kind="ExternalInput")
    gtab_d = nc.dram_tensor("gtab", [128, 64], f32, kind="ExternalInput")
    stab_d = nc.dram_tensor("stab", [128, 64], f32, kind="ExternalInput")
    ubias_d = nc.dram_tensor("ubias", [128, 64], f32, kind="ExternalInput")
    idx_d = nc.dram_tensor("idx", [128, 64], i32, kind="ExternalInput")
    outT_d = nc.dram_tensor("outT", [O, rc], bf16, kind="ExternalOutput")

    TWO_PI = float(2.0 * math.pi)

    with tile.TileContext(nc) as tc:
        with (
            tc.tile_pool(name="consts", bufs=1) as cpool,
            tc.tile_pool(name="ab", bufs=1) as abpool,
            tc.tile_pool(name="gb", bufs=6) as gbpool,
            tc.tile_pool(name="ct", bufs=1) as ctpool,
            tc.tile_pool(name="dt", bufs=3) as dtpool,
            tc.tile_pool(name="ep", bufs=4) as eppool,
            tc.tile_pool(name="ps", bufs=4, space="PSUM") as pspool,
            tc.tile_pool(name="dram", bufs=1, space="DRAM") as drampool,
        ):
            w1 = cpool.tile([128, 16 * 128], f32)
            nc.sync.dma_start(w1[:], w1_d[:])
            h128f = cpool.tile([128, 128], f32)
            nc.sync.dma_start(h128f[:], h128f_d[:])
            hd64 = cpool.tile([128, 128], bf16)
            nc.sync.dma_start(hd64[:], hd64_d[:])
            gtab = cpool.tile([128, 64], f32)
            nc.sync.dma_start(gtab[:], gtab_d[:])
            stab = cpool.tile([128, 64], f32)
            nc.sync.dma_start(stab[:], stab_d[:])
            ubias = cpool.tile([128, 64], f32)
            nc.sync.dma_start(ubias[:], ubias_d[:])
            idx = cpool.tile([128, 64], i32)
            nc.sync.dma_start(idx[:], idx_d[:])

            hbxt = drampool.tile([D, rc], f32)

            # ---- P1: load x (one DMA; SBUF side plain, DRAM side fancy) ----
            xbuf = cpool.tile([128, 16 * rc], f32)
            for lc in range(4):
                nc.sync.dma_start(
                    xbuf[:, 4 * lc * rc : 4 * (lc + 1) * rc],
                    xt_d[4 * lc : 4 * (lc + 1), :, :].rearrange(
                        "l p r -> p l r"
                    ),
                )

            # stage A: 16 fp32 matmuls, pairs into [128, 2*rc] PSUM banks
            abuf = abpool.tile([128, 16 * rc], f32, tag="a")
            bbuf = abpool.tile([128, 16 * rc], f32, tag="b")
            for q in range(8):
                ps = pspool.tile([128, 2 * rc], f32, tag="ps")
                for s in range(2):
                    lo = 2 * q + s
                    nc.tensor.matmul(
                        out=ps[:, s * rc : (s + 1) * rc],
                        lhsT=w1[:, lo * 128 : (lo + 1) * 128],
                        rhs=xbuf[:, lo * rc : (lo + 1) * rc],
                        start=True,
                        stop=True,
                    )
                # H16 level h=1: TT may read only one PSUM input, so copy
                # the odd half to SBUF first (ACT), then add/sub from PSUM.
                odd = bbuf[:, (2 * q) * rc : (2 * q + 1) * rc]
                nc.scalar.activation(
                    out=odd,
                    in_=ps[:, rc : 2 * rc],
                    func=mybir.ActivationFunctionType.Copy,
                )
                nc.vector.tensor_tensor(
                    out=abuf[:, (2 * q) * rc : (2 * q + 1) * rc],
                    in0=ps[:, 0:rc],
                    in1=odd,
                    op=mybir.AluOpType.add,
                )
                nc.vector.tensor_tensor(
                    out=abuf[:, (2 * q + 1) * rc : (2 * q + 2) * rc],
                    in0=ps[:, 0:rc],
                    in1=odd,
                    op=mybir.AluOpType.subtract,
                )
            # levels h=2,4,8: merged wide ops, ping-pong abuf <-> bbuf
            src, dst = abuf, bbuf
            for h in (2, 4, 8):
                w = h * rc
                for i in range(0, 16, 2 * h):
                    a0 = src[:, i * rc : i * rc + w]
                    a1 = src[:, (i + h) * rc : (i + h) * rc + w]
                    nc.vector.tensor_tensor(
                        out=dst[:, i * rc : i * rc + w],
                        in0=a0, in1=a1, op=mybir.AluOpType.add,
                    )
                    nc.gpsimd.tensor_tensor(
                        out=dst[:, (i + h) * rc : (i + h) * rc + w],
                        in0=a0, in1=a1, op=mybir.AluOpType.subtract,
                    )
                src, dst = dst, src
            hb = src  # tile lo holds feature f' = hi*16 + lo on partition hi

            # hbxt: one DMA, hb[hi, lo*rc+r] -> DRAM row lo*128 + hi
            nc.sync.dma_start(
                hbxt[:].rearrange("(l h) r -> h l r", l=16),
                hb[:],
            )

            # ---- P2: gather + G-scale + bf16-split stage C ----
            # ctbuf free layout per lo-tile: [hi bf16 (rc) | lo bf16 (rc)]
            ctbuf = ctpool.tile([128, 64 * 2 * rc], bf16)
            ct4 = ctbuf[:].rearrange("p (l t r) -> p l t r", l=64, t=2)
            ctd = drampool.tile([O, 2 * rc], bf16)  # row hi*64+lo, [hi|lo]
            ctd3 = ctd[:].rearrange("(h l) v -> h l v", l=64)
            for q in range(32):
                ps = pspool.tile([128, 2 * rc], f32, tag="ps")
                p32 = gbpool.tile([128, 2 * rc], f32, tag="p32")
                for s in range(2):
                    lo = 2 * q + s
                    g = gbpool.tile([128, rc], f32, tag="g")
                    nc.gpsimd.indirect_dma_start(
                        out=g[:],
                        out_offset=None,
                        in_=hbxt[:, :],
                        in_offset=bass.IndirectOffsetOnAxis(
                            ap=idx[:, lo : lo + 1], axis=0
                        ),
                    )
                    # p32 = g * G (fp32, ACT)
                    nc.scalar.activation(
                        out=p32[:, s * rc : (s + 1) * rc],
                        in_=g[:],
                        func=mybir.ActivationFunctionType.Copy,
                        scale=gtab[:, lo : lo + 1],
                    )
                # one merged fp32 matmul per PSUM bank (halves LDWEIGHTS)
                nc.tensor.matmul(
                    out=ps[:],
                    lhsT=h128f[:],
                    rhs=p32[:],
                    start=True,
                    stop=True,
                )
                lo0 = 2 * q
                # hi: bf16 round of psum pair (strided out), on ACT
                nc.scalar.activation(
                    out=ct4[:, lo0 : lo0 + 2, 0, :],
                    in_=ps[:],
                    func=mybir.ActivationFunctionType.Copy,
                )
                # lo: psum - hi (strided bf16 out), on DVE
                nc.vector.tensor_tensor(
                    out=ct4[:, lo0 : lo0 + 2, 1, :],
                    in0=ps[:],
                    in1=ct4[:, lo0 : lo0 + 2, 0, :],
                    op=mybir.AluOpType.subtract,
                )
                if q % 4 == 3:
                    # ship lo-group [8gc, 8gc+8) to DRAM (SP idle in P2)
                    gc = q // 4
                    nc.sync.dma_start(
                        ctd3[:, 8 * gc : 8 * gc + 8, :],
                        ctbuf[:, (8 * gc) * 2 * rc : (8 * gc + 8) * 2 * rc],
                    )

            # ---- P3: dt octet reads + stage D + epilogue ----
            # ctd row hi*64+lo; dt tile pt = ctd rows [pt*128, (pt+1)*128)
            # (partition b*64+lo <-> row (2pt+b)*64+lo). Octet read g:
            # [128 rows x 8 blocks x 2rc] -> dt8 [128, 8*2rc] plain.
            ctd4 = ctd[:].rearrange("(g b p) v -> g p b v", g=8, b=8)
            dt8s = []
            for g8 in range(8):
                dt8 = dtpool.tile([128, 8 * 2 * rc], bf16, tag="dt")
                nc.sync.dma_start(dt8[:], ctd4[g8, :, :, :])
                dt8s.append(dt8)
            for k in range(32):
                psd = pspool.tile([128, 2 * rc], f32, tag="ps")
                dt8 = dt8s[(2 * k) // 8]
                b0 = (2 * k) % 8
                dt8v = dt8[:].rearrange("p (b q r) -> p b q r", b=8, q=2)
                for part in range(2):  # hi then lo halves, accumulated
                    nc.tensor.matmul(
                        out=psd[:],
                        lhsT=hd64[:],
                        rhs=dt8v[:, b0 : b0 + 2, part, :],
                        start=(part == 0),
                        stop=(part == 1),
                    )
                # z = psd*stab + ubias per tile (per-tile scalars)
                z = eppool.tile([128, 2 * rc], f32, tag="z")
                for t in range(2):
                    pt = 2 * k + t
                    nc.vector.tensor_scalar(
                        out=z[:, t * rc : (t + 1) * rc],
                        in0=psd[:, t * rc : (t + 1) * rc],
                        scalar1=stab[:, pt : pt + 1],
                        scalar2=ubias[:, pt : pt + 1],
                        op0=mybir.AluOpType.mult,
                        op1=mybir.AluOpType.add,
                    )
                # range reduce (wide): k32 = int32(z) RNE (ACT), d = z - k32
                k32 = eppool.tile([128, 2 * rc], i32, tag="k")
                nc.scalar.activation(
                    out=k32[:],
                    in_=z[:],
                    func=mybir.ActivationFunctionType.Copy,
                )
                dred = eppool.tile([128, 2 * rc], f32, tag="d")
                nc.vector.tensor_tensor(
                    out=dred[:], in0=z[:], in1=k32[:],
                    op=mybir.AluOpType.subtract,
                )
                # out = Sin(2*pi*d) in bf16 on ACT
                so = eppool.tile([128, 2 * rc], bf16, tag="so")
                nc.scalar.activation(
                    out=so[:],
                    in_=dred[:],
                    func=mybir.ActivationFunctionType.Sin,
                    scale=TWO_PI,
                )
                # rows [256k, 256k+256) of outT; DRAM side fancy, SBUF plain
                nc.sync.dma_start(
                    outT_d[256 * k : 256 * (k + 1), :].rearrange(
                        "(t p) r -> p t r", t=2
                    ),
                    so[:],
                )

    nc.compile()
    return nc


def host_prep(x, B, G, S, P, U):
    xr = np.ascontiguousarray(x.reshape(R, D).astype(np.float32))
    H128 = _hadamard(128)
    H64 = _hadamard(64)

    # w1[hi, lo*128 + m] = B[hi*16+lo] * H128[hi, m]
    Bm = B.reshape(128, 16).astype(np.float32)
    w1 = (Bm[:, :, None] * H128[:, None, :]).reshape(128, 16 * 128)
    hd64 = np.zeros((128, 128), dtype=np.float32)
    hd64[:64, :64] = H64
    hd64[64:, 64:] = H64

    gtab = np.ascontiguousarray(G.reshape(128, 64).astype(np.float32))
    # z = psd*stab + ubias in "periods"; sin(2*pi*z) via RNE range reduce.
    stab = np.ascontiguousarray(
        (S.astype(np.float64) / (math.sqrt(O) * 2.0 * math.pi))
        .astype(np.float32)
        .reshape(64, 128)
        .T
    )
    ub = U.astype(np.float64) + 0.25
    ubias = np.ascontiguousarray(ub.astype(np.float32).reshape(64, 128).T)

    fp = (P.astype(np.int64) % D).reshape(128, 64)
    idx = ((fp % 16) * 128 + (fp // 16)).astype(np.int32)

    import ml_dtypes

    consts = dict(
        w1=np.ascontiguousarray(w1),
        h128=np.ascontiguousarray(H128),
        hd64=hd64.astype(ml_dtypes.bfloat16),
        gtab=gtab,
        stab=stab,
        ubias=ubias,
        idx=np.ascontiguousarray(idx),
    )

    shards = []
    for c in range(N_CORES):
        xs = xr[c * RC : (c + 1) * RC]  # (RC, D)
        xt = np.ascontiguousarray(
            xs.T.reshape(128, 16, RC).transpose(1, 0, 2)
        )  # xt[lo, hi, r] = xs[r, hi*16+lo]
        shards.append(xt)
    return consts, shards


def assemble(core_outs):
    """core_outs: list of (O, RC) bf16 arrays -> full (R, O) f32 output."""
    out = np.empty((R, O), dtype=np.float32)
    for c, ot in enumerate(core_outs):
        out[c * RC : (c + 1) * RC, :] = ot.astype(np.float32).T
    out *= np.float32(1.0 / 64.0)  # sqrt(2/O) = 2^-6, exact
    return out


def kernel(x, B, G, S, P, U):
    from concourse.bass_utils import run_bass_kernel_spmd

    if "nc" not in _CACHE:
        _CACHE["nc"] = _build_nc(RC)
    nc = _CACHE["nc"]

    consts, shards = host_prep(x, B, G, S, P, U)
    in_maps = [dict(consts, xt=shards[c]) for c in range(N_CORES)]

    res = run_bass_kernel_spmd(
        nc,
        in_maps,
        core_ids=list(range(N_CORES)),
        trace=TRACE,
        **TRACE_KW,
    )
    _CACHE["last_result"] = res
    return assemble([r["outT"] for r in res.results])



# revision 22
# speedup vs baseline: 1.2729x; 1.0296x over previous
"""Fastfood layer (nn_BIG_Fastfood_Layer) Trainium2 Bass kernel, v2.

Math (reference):
    xr = x.reshape(2048, 2048)                       # (R, D)
    HBx = fwht_2048(xr * B)                          # (R, D)
    GPHBx[:, j] = HBx[:, P[j] % 2048] * G[j]         # (R, O) tile+permute+scale
    HG = fwht_8192(GPHBx)                            # (R, O)
    out = cos(HG * S / sqrt(O) + 2*pi*U) * sqrt(2/O) # (R, O)

v2 design (data-parallel rows, 8 cores, rc = 256 rows/core; feature-major):

  P1: one DMA loads x transposed; FWHT_2048 = H_128 (x) H_16:
      stage A = 16 fp32 matmuls (lhsT = diag(B_lo) @ H128) packed in pairs
      into [128,512] PSUM banks; H_16 butterfly: level 1 reads PSUM pairs,
      levels 2/4/8 merged wide ops, add/sub split DVE/Pool; one DMA writes
      hbxt to DRAM rows lo*128 + hi.
  P2: 64 single-index indirect gathers (Pool SWDGE paces this phase)
      realize the P-permutation + x4 tile + retile; G-scale per tile
      (ACT/DVE); stage C = 64 fp32 matmuls lhsT = H128 (PE hidden under
      Pool); PSUM pairs split to bf16 hi/lo interleaved in ctbuf
      (hi-copy on ACT, lo = psum - hi on DVE).
  P3: linear DRAM round trip realizes the flip; stage D per tile two
      -> dt (part = lo + 64*(hi'&1), tile=hi'>>1); stage D = per tile two
      bf16 matmuls (hi + lo PSUM-accumulated) with lhsT = blockdiag(H64);
      epilogue z = psum*stab + ubias (TS per tile), k = int32(z) RNE,
      d = z - k, out = Sin(2*pi*d) in bf16; host up-casts and applies
      sqrt(2/O) = 2^-6 (exact).
"""

import math

import numpy as np

D = 2048
O = 8192
R = 2048
N_CORES = 8
RC = R // N_CORES  # 256 rows per core

TRACE = False
TRACE_KW = {}

_CACHE = {}


def _hadamard(n):
    h = np.array([[1.0]], dtype=np.float32)
    while h.shape[0] < n:
        h = np.block([[h, h], [h, -h]])
    return h.astype(np.float32)


def _build_nc(rc):
    import concourse.bass as bass
    import concourse.mybir as mybir
    import concourse.tile as tile
    from concourse import bacc

    f32 = mybir.dt.float32
    bf16 = mybir.dt.bfloat16
    i32 = mybir.dt.int32
    nc = bacc.Bacc("TRN2", target_bir_lowering=False)

    xt_d = nc.dram_tensor("xt", [16, 128, rc], f32, kind="ExternalInput")
    w1_d = nc.dram_tensor("w1", [128, 16 * 128], f32, kind="ExternalInput")
    h128f_d = nc.dram_tensor("h128", [128, 128], f32, kind="ExternalInput")
    hd64_d = nc.dram_tensor("hd64", [128, 128], bf16, kind="ExternalInput")
    gtab_d = nc.dram_tensor("gtab", [128, 64], f32, kind="ExternalInput")
    stab_d = nc.dram_tensor("stab", [128, 64], f32, kind="ExternalInput")
    ubias_d = nc.dram_tensor("ubias", [128, 64], f32, kind="ExternalInput")
    idx_d = nc.dram_tensor("idx", [128, 64], i32, kind="ExternalInput")
    outT_d = nc.dram_tensor("outT", [O, rc], bf16, kind="ExternalOutput")

    TWO_PI = float(2.0 * math.pi)

    with tile.TileContext(nc) as tc:
        with (
            tc.tile_pool(name="consts", bufs=1) as cpool,
            tc.tile_pool(name="ab", bufs=1) as abpool,
            tc.tile_pool(name="gb", bufs=6) as gbpool,
            tc.tile_pool(name="ct", bufs=1) as ctpool,
            tc.tile_pool(name="dt", bufs=3) as dtpool,
            tc.tile_pool(name="ep", bufs=4) as eppool,
            tc.tile_pool(name="ps", bufs=6, space="PSUM") as pspool,
            tc.tile_pool(name="dram", bufs=1, space="DRAM") as drampool,
        ):
            w1 = cpool.tile([128, 16 * 128], f32)
            nc.sync.dma_start(w1[:], w1_d[:])
            h128f = cpool.tile([128, 128], f32)
            nc.sync.dma_start(h128f[:], h128f_d[:])
            hd64 = cpool.tile([128, 128], bf16)
            nc.sync.dma_start(hd64[:], hd64_d[:])
            gtab = cpool.tile([128, 64], f32)
            nc.sync.dma_start(gtab[:], gtab_d[:])
            stab = cpool.tile([128, 64], f32)
            nc.sync.dma_start(stab[:], stab_d[:])
            ubias = cpool.tile([128, 64], f32)
            nc.sync.dma_start(ubias[:], ubias_d[:])
            idx = cpool.tile([128, 64], i32)
            nc.sync.dma_start(idx[:], idx_d[:])

            hbxt = drampool.tile([D, rc], f32)

            # ---- P1: load x (one DMA; SBUF side plain, DRAM side fancy) ----
            xbuf = cpool.tile([128, 16 * rc], f32)
            for lc in range(4):
                nc.sync.dma_start(
                    xbuf[:, 4 * lc * rc : 4 * (lc + 1) * rc],
                    xt_d[4 * lc : 4 * (lc + 1), :, :].rearrange(
                        "l p r -> p l r"
                    ),
                )

            # stage A: 16 fp32 matmuls, pairs into [128, 2*rc] PSUM banks
            abuf = abpool.tile([128, 16 * rc], f32, tag="a")
            bbuf = abpool.tile([128, 16 * rc], f32, tag="b")
            for q in range(8):
                ps = pspool.tile([128, 2 * rc], f32, tag="ps")
                for s in range(2):
                    lo = 2 * q + s
                    nc.tensor.matmul(
                        out=ps[:, s * rc : (s + 1) * rc],
                        lhsT=w1[:, lo * 128 : (lo + 1) * 128],
                        rhs=xbuf[:, lo * rc : (lo + 1) * rc],
                        start=True,
                        stop=True,
                    )
                # H16 level h=1: TT may read only one PSUM input, so copy
                # the odd half to SBUF first (ACT), then add/sub from PSUM.
                odd = bbuf[:, (2 * q) * rc : (2 * q + 1) * rc]
                nc.scalar.activation(
                    out=odd,
                    in_=ps[:, rc : 2 * rc],
                    func=mybir.ActivationFunctionType.Copy,
                )
                nc.vector.tensor_tensor(
                    out=abuf[:, (2 * q) * rc : (2 * q + 1) * rc],
                    in0=ps[:, 0:rc],
                    in1=odd,
                    op=mybir.AluOpType.add,
                )
                nc.vector.tensor_tensor(
                    out=abuf[:, (2 * q + 1) * rc : (2 * q + 2) * rc],
                    in0=ps[:, 0:rc],
                    in1=odd,
                    op=mybir.AluOpType.subtract,
                )
            # levels h=2,4,8: merged wide ops, ping-pong abuf <-> bbuf
            src, dst = abuf, bbuf
            for h in (2, 4, 8):
                w = h * rc
                for i in range(0, 16, 2 * h):
                    a0 = src[:, i * rc : i * rc + w]
                    a1 = src[:, (i + h) * rc : (i + h) * rc + w]
                    nc.vector.tensor_tensor(
                        out=dst[:, i * rc : i * rc + w],
                        in0=a0, in1=a1, op=mybir.AluOpType.add,
                    )
                    nc.gpsimd.tensor_tensor(
                        out=dst[:, (i + h) * rc : (i + h) * rc + w],
                        in0=a0, in1=a1, op=mybir.AluOpType.subtract,
                    )
                src, dst = dst, src
            hb = src  # tile lo holds feature f' = hi*16 + lo on partition hi

            # hbxt: one DMA, hb[hi, lo*rc+r] -> DRAM row lo*128 + hi
            nc.sync.dma_start(
                hbxt[:].rearrange("(l h) r -> h l r", l=16),
                hb[:],
            )

            # ---- P2: gather + G-scale + bf16-split stage C ----
            # ctbuf free layout per lo-tile: [hi bf16 (rc) | lo bf16 (rc)]
            ctbuf = ctpool.tile([128, 64 * 2 * rc], bf16)
            ct4 = ctbuf[:].rearrange("p (l t r) -> p l t r", l=64, t=2)
            ctd = drampool.tile([O, 2 * rc], bf16)  # row hi*64+lo, [hi|lo]
            ctd3 = ctd[:].rearrange("(h l) v -> h l v", l=64)
            for q in range(32):
                ps = pspool.tile([128, 2 * rc], f32, tag="ps")
                p32 = gbpool.tile([128, 2 * rc], f32, tag="p32")
                for s in range(2):
                    lo = 2 * q + s
                    g = gbpool.tile([128, rc], f32, tag="g")
                    nc.gpsimd.indirect_dma_start(
                        out=g[:],
                        out_offset=None,
                        in_=hbxt[:, :],
                        in_offset=bass.IndirectOffsetOnAxis(
                            ap=idx[:, lo : lo + 1], axis=0
                        ),
                    )
                    # p32 = g * G (fp32, ACT)
                    nc.scalar.activation(
                        out=p32[:, s * rc : (s + 1) * rc],
                        in_=g[:],
                        func=mybir.ActivationFunctionType.Copy,
                        scale=gtab[:, lo : lo + 1],
                    )
                # one merged fp32 matmul per PSUM bank (halves LDWEIGHTS)
                nc.tensor.matmul(
                    out=ps[:],
                    lhsT=h128f[:],
                    rhs=p32[:],
                    start=True,
                    stop=True,
                )
                lo0 = 2 * q
                # hi: bf16 round of psum pair (strided out), on ACT
                nc.scalar.activation(
                    out=ct4[:, lo0 : lo0 + 2, 0, :],
                    in_=ps[:],
                    func=mybir.ActivationFunctionType.Copy,
                )
                # lo: psum - hi (strided bf16 out), on DVE
                nc.vector.tensor_tensor(
                    out=ct4[:, lo0 : lo0 + 2, 1, :],
                    in0=ps[:],
                    in1=ct4[:, lo0 : lo0 + 2, 0, :],
                    op=mybir.AluOpType.subtract,
                )
                if q % 4 == 3:
                    # ship lo-group [8gc, 8gc+8) to DRAM (SP idle in P2)
                    gc = q // 4
                    nc.sync.dma_start(
                        ctd3[:, 8 * gc : 8 * gc + 8, :],
                        ctbuf[:, (8 * gc) * 2 * rc : (8 * gc + 8) * 2 * rc],
                    )

            # ---- P3: dt octet reads + stage D + epilogue ----
            # ctd row hi*64+lo; dt tile pt = ctd rows [pt*128, (pt+1)*128)
            # (partition b*64+lo <-> row (2pt+b)*64+lo). Octet read g:
            # [128 rows x 8 blocks x 2rc] -> dt8 [128, 8*2rc] plain.
            ctd4 = ctd[:].rearrange("(g b p) v -> g p b v", g=8, b=8)
            dt8s = []
            for g8 in range(8):
                dt8 = dtpool.tile([128, 8 * 2 * rc], bf16, tag="dt")
                nc.sync.dma_start(dt8[:], ctd4[g8, :, :, :])
                dt8s.append(dt8)
            for k in range(32):
                psd = pspool.tile([128, 2 * rc], f32, tag="ps")
                dt8 = dt8s[(2 * k) // 8]
                b0 = (2 * k) % 8
                dt8v = dt8[:].rearrange("p (b q r) -> p b q r", b=8, q=2)
                for part in range(2):  # hi then lo halves, accumulated
                    nc.tensor.matmul(
                        out=psd[:],
                        lhsT=hd64[:],
                        rhs=dt8v[:, b0 : b0 + 2, part, :],
                        start=(part == 0),
                        stop=(part == 1),
                    )
                # z = psd*stab + ubias per tile (per-tile scalars)
                z = eppool.tile([128, 2 * rc], f32, tag="z")
                for t in range(2):
                    pt = 2 * k + t
                    nc.vector.tensor_scalar(
                        out=z[:, t * rc : (t + 1) * rc],
                        in0=psd[:, t * rc : (t + 1) * rc],
                        scalar1=stab[:, pt : pt + 1],
                        scalar2=ubias[:, pt : pt + 1],
                        op0=mybir.AluOpType.mult,
                        op1=mybir.AluOpType.add,
                    )
                # range reduce (wide): k32 = int32(z) RNE (ACT), d = z - k32
                k32 = eppool.tile([128, 2 * rc], i32, tag="k")
                nc.scalar.activation(
                    out=k32[:],
                    in_=z[:],
                    func=mybir.ActivationFunctionType.Copy,
                )
                dred = eppool.tile([128, 2 * rc], f32, tag="d")
                nc.vector.tensor_tensor(
                    out=dred[:], in0=z[:], in1=k32[:],
                    op=mybir.AluOpType.subtract,
                )
                # out = Sin(2*pi*d) in bf16 on ACT
                so = eppool.tile([128, 2 * rc], bf16, tag="so")
                nc.scalar.activation(
                    out=so[:],
                    in_=dred[:],
                    func=mybir.ActivationFunctionType.Sin,
                    scale=TWO_PI,
                )
                # rows [256k, 256k+256) of outT; DRAM side fancy, SBUF plain
                nc.sync.dma_start(
                    outT_d[256 * k : 256 * (k + 1), :].rearrange(
                        "(t p) r -> p t r", t=2
                    ),
                    so[:],
                )

    nc.compile()
    return nc


def host_prep(x, B, G, S, P, U):
    xr = np.ascontiguousarray(x.reshape(R, D).astype(np.float32))
    H128 = _hadamard(128)
    H64 = _hadamard(64)

    # w1[hi, lo*128 + m] = B[hi*16+lo] * H128[hi, m]
    Bm = B.reshape(128, 16).astype(np.float32)
    w1 = (Bm[:, :, None] * H128[:, None, :]).reshape(128, 16 * 128)
    hd64 = np.zeros((128, 128), dtype=np.float32)
    hd64[:64, :64] = H64
    hd64[64:, 64:] = H64

    gtab = np.ascontiguousarray(G.reshape(128, 64).astype(np.float32))
    # z = psd*stab + ubias in "periods"; sin(2*pi*z) via RNE range reduce.
    stab = np.ascontiguousarray(
        (S.astype(np.float64) / (math.sqrt(O) * 2.0 * math.pi))
        .astype(np.float32)
        .reshape(64, 128)
        .T
    )
    ub = U.astype(np.float64) + 0.25
    ubias = np.ascontiguousarray(ub.astype(np.float32).reshape(64, 128).T)

    fp = (P.astype(np.int64) % D).reshape(128, 64)
    idx = ((fp % 16) * 128 + (fp // 16)).astype(np.int32)

    import ml_dtypes

    consts = dict(
        w1=np.ascontiguousarray(w1),
        h128=np.ascontiguousarray(H128),
        hd64=hd64.astype(ml_dtypes.bfloat16),
        gtab=gtab,
        stab=stab,
        ubias=ubias,
        idx=np.ascontiguousarray(idx),
    )

    shards = []
    for c in range(N_CORES):
        xs = xr[c * RC : (c + 1) * RC]  # (RC, D)
        xt = np.ascontiguousarray(
            xs.T.reshape(128, 16, RC).transpose(1, 0, 2)
        )  # xt[lo, hi, r] = xs[r, hi*16+lo]
        shards.append(xt)
    return consts, shards


def assemble(core_outs):
    """core_outs: list of (O, RC) bf16 arrays -> full (R, O) f32 output."""
    out = np.empty((R, O), dtype=np.float32)
    for c, ot in enumerate(core_outs):
        out[c * RC : (c + 1) * RC, :] = ot.astype(np.float32).T
    out *= np.float32(1.0 / 64.0)  # sqrt(2/O) = 2^-6, exact
    return out


def kernel(x, B, G, S, P, U):
    from concourse.bass_utils import run_bass_kernel_spmd

    if "nc" not in _CACHE:
        _CACHE["nc"] = _build_nc(RC)
    nc = _CACHE["nc"]

    consts, shards = host_prep(x, B, G, S, P, U)
    in_maps = [dict(consts, xt=shards[c]) for c in range(N_CORES)]

    res = run_bass_kernel_spmd(
        nc,
        in_maps,
        core_ids=list(range(N_CORES)),
        trace=TRACE,
        **TRACE_KW,
    )
    _CACHE["last_result"] = res
    return assemble([r["outT"] for r in res.results])

